# revision 1
# baseline (speedup 1.0000x reference)
"""Trainium2 Bass kernel for nn_ConceptLayer (B=8, S=2048, E=128).

out[b,s,c] = LN( einsum('sa,sp,cap->sc', h[b], s_seq[b], W) + h[b] )
  h = x @ dense_w + dense_b
  s_seq = decayed prefix sum of h along s (s_i = (s_{i-1}+h_{i-1})/1.2)

Sharding: data-parallel over batch, one sample per NeuronCore (8 cores).
concept_map replicated (host-pretransposed to [p, (a,c)] bf16); x is
host-pretransposed to xT [e, t] so no on-device transposes are needed.

Per-core pipeline:
  1. hT-chunk matmuls (dense_w.T @ xT, fp32); PSUM->SBUF copy applies
     (+bias)*1/DECAY producing the scan feed hTp (shifted one col)
  2. s_seq^T via tensor_tensor_scan (state = state/d + (h+b)/d) in fp32,
     downcast to bf16 (sTmm) for the tensor-engine operand
  3. h natural layout per 128-token block: matmul(lhsT=xT slice, rhs=dense_w)
     plus a K=1 ones-x-bias matmul into the same PSUM
  4. per block: acc = h_blk (residual); 32 matmuls N=512
     (lhsT = sT block stationary, rhs = W2 slices) -> Y in PSUM;
     fused axpy acc += Y[:,a-slice] * h[:,a] (scalar_tensor_tensor)
  5. LayerNorm (bn_stats/bn_aggr, sqrt, reciprocal) + gamma/beta, DMA out
"""

import os
import sys

import numpy as np

for _p in ("/opt/trn_rl_repo",):
    if _p not in sys.path and os.path.isdir(_p):
        sys.path.insert(0, _p)

import concourse.bass as bass
import concourse.bacc as bacc
import concourse.tile as tile
from concourse import mybir
from concourse.bass_utils import run_bass_kernel_spmd

B, S, E = 8, 2048, 128
DECAY = 1.2
LN_EPS = 1e-3
NBLK = S // 128          # 16 token blocks per core
NCHUNK = (E * E) // 512  # 32 matmul chunks of 512 (a,c) columns per block
F32 = mybir.dt.float32
BF16 = mybir.dt.bfloat16
NPBF16 = mybir.dt.np(BF16)

_CACHE = {}
LAST_RESULT = None  # BassKernelResults of the most recent run (for test.py)


def _build_nc():
    nc = bacc.Bacc(None, target_bir_lowering=False)

    xT_d = nc.declare_dram_parameter("xT", [E, S], BF16, isOutput=False)
    dw_d = nc.declare_dram_parameter("dense_w", [E, E], BF16, isOutput=False)
    bc_d = nc.declare_dram_parameter("b_col", [E, 1], F32, isOutput=False)
    br_d = nc.declare_dram_parameter("b_row", [1, E], BF16, isOutput=False)
    w2_d = nc.declare_dram_parameter("w2", [E, E * E], BF16, isOutput=False)
    gam_d = nc.declare_dram_parameter("gamma_rep", [128, E], F32, isOutput=False)
    bet_d = nc.declare_dram_parameter("beta_rep", [128, E], F32, isOutput=False)
    out_d = nc.declare_dram_parameter("out", [S, E], F32, isOutput=True)

    with tile.TileContext(nc) as tc:
        with (
            tc.tile_pool(name="singles", bufs=1) as singles,
            tc.tile_pool(name="blk", bufs=2) as blk,
            tc.tile_pool(name="small", bufs=4) as small,
            tc.tile_pool(name="h_ps", bufs=2, space="PSUM") as h_ps,
            tc.tile_pool(name="hn_ps", bufs=2, space="PSUM") as hn_ps,
            tc.tile_pool(name="y_ps", bufs=4, space="PSUM") as y_ps,
        ):
            # ---- resident tensors ----
            w2_sb = singles.tile([E, E * E], BF16)
            nc.sync.dma_start(out=w2_sb[:], in_=w2_d[:])
            dw_sb = singles.tile([E, E], BF16)
            nc.sync.dma_start(out=dw_sb[:], in_=dw_d[:])
            bcol = singles.tile([E, 1], F32)
            nc.sync.dma_start(out=bcol[:], in_=bc_d[:])
            brow = singles.tile([1, E], BF16)
            nc.sync.dma_start(out=brow[:], in_=br_d[:])
            gam_sb = singles.tile([128, E], F32)
            nc.sync.dma_start(out=gam_sb[:], in_=gam_d[:])
            bet_sb = singles.tile([128, E], F32)
            nc.sync.dma_start(out=bet_sb[:], in_=bet_d[:])
            xT = singles.tile([E, S], BF16)
            nc.sync.dma_start(out=xT[:], in_=xT_d[:])

            eps_t = singles.tile([128, 1], F32)
            nc.vector.memset(eps_t[:], LN_EPS)
            dinv = singles.tile([128, 512], F32)
            nc.vector.memset(dinv[:], 1.0 / DECAY)
            ones1 = singles.tile([1, 128], BF16)
            nc.vector.memset(ones1[:], 1.0)

            hTp = singles.tile([E, S + 1], F32)   # col j+1 = (h_j+b)/d, col0 = 0
            sT32 = singles.tile([E, S], F32)
            sTmm = singles.tile([E, S], BF16)
            h_sb = singles.tile([128, NBLK, E], F32)  # h natural, blocked

            nc.vector.memset(hTp[:, 0:1], 0.0)

            # ---- 1. hT chunks = dense_w.T @ xT; scan feed with (+b)/d ----
            for q in range(4):
                hp = h_ps.tile([E, 512], F32)
                nc.tensor.matmul(hp[:], dw_sb[:], xT[:, q * 512:(q + 1) * 512],
                                 start=True, stop=True)
                nc.vector.tensor_scalar(
                    hTp[:, q * 512 + 1:(q + 1) * 512 + 1], hp[:], bcol[:],
                    1.0 / DECAY, mybir.AluOpType.add, mybir.AluOpType.mult)

            # ---- 2. decay prefix scan -> sT ----
            for q in range(4):
                lo, hi = q * 512, (q + 1) * 512
                init = 0.0 if q == 0 else sT32[:, lo - 1:lo]
                nc.vector.tensor_tensor_scan(
                    sT32[:, lo:hi], dinv[:], hTp[:, lo:hi], init,
                    mybir.AluOpType.mult, mybir.AluOpType.add)
                nc.gpsimd.tensor_copy(out=sTmm[:, lo:hi], in_=sT32[:, lo:hi])

            # ---- 3. h natural layout: h_blk = xT_blk.T @ dense_w + 1s*b ----
            for g in range(NBLK):
                hp = hn_ps.tile([128, E], F32)
                nc.tensor.matmul(hp[:], xT[:, g * 128:(g + 1) * 128], dw_sb[:],
                                 start=True, stop=False)
                nc.tensor.matmul(hp[:], ones1[:], brow[:],
                                 start=False, stop=True)
                nc.scalar.copy(out=h_sb[:, g, :], in_=hp[:])

            # ---- 4+5. main einsum + residual + LN per block ----
            for g in range(NBLK):
                acc = blk.tile([128, E], F32)
                nc.vector.tensor_copy(out=acc[:], in_=h_sb[:, g, :])  # residual
                sT_blk = sTmm[:, g * 128:(g + 1) * 128]
                for j in range(NCHUNK):
                    yp = y_ps.tile([128, 512], F32)
                    nc.tensor.matmul(yp[:], sT_blk,
                                     w2_sb[:, j * 512:(j + 1) * 512],
                                     start=True, stop=True)
                    for k in range(4):
                        a = 4 * j + k
                        nc.vector.scalar_tensor_tensor(
                            acc[:], yp[:, k * 128:(k + 1) * 128],
                            h_sb[:, g, a:a + 1], acc[:],
                            mybir.AluOpType.mult, mybir.AluOpType.add)

                stats = small.tile([128, 6], F32)
                nc.vector.bn_stats(out=stats[:], in_=acc[:])
                mv = small.tile([128, 2], F32)
                nc.vector.bn_aggr(out=mv[:], in_=stats[:])
                std = small.tile([128, 1], F32)
                nc.scalar.activation(out=std[:], in_=mv[:, 1:2],
                                     func=mybir.ActivationFunctionType.Sqrt,
                                     bias=eps_t[:], scale=1.0)
                rstd = small.tile([128, 1], F32)
                nc.vector.reciprocal(out=rstd[:], in_=std[:])
                nrm = blk.tile([128, E], F32)
                nc.vector.tensor_scalar(
                    nrm[:], acc[:], mv[:, 0:1], rstd[:],
                    mybir.AluOpType.subtract, mybir.AluOpType.mult)
                nc.vector.tensor_mul(nrm[:], nrm[:], gam_sb[:])
                nc.vector.tensor_add(nrm[:], nrm[:], bet_sb[:])
                nc.sync.dma_start(out=out_d[g * 128:(g + 1) * 128, :], in_=nrm[:])

    nc.compile()
    return nc


def _get_nc():
    if "nc" not in _CACHE:
        _CACHE["nc"] = _build_nc()
    return _CACHE["nc"]


def kernel(x, dense_w, dense_b, concept_map, ln_gamma, ln_beta):
    global LAST_RESULT
    x = np.asarray(x, dtype=np.float32)
    dense_w = np.ascontiguousarray(np.asarray(dense_w, dtype=np.float32))
    b = np.asarray(dense_b, dtype=np.float32)
    w2 = np.ascontiguousarray(
        np.transpose(np.asarray(concept_map, dtype=np.float32), (2, 1, 0))
    ).reshape(E, E * E).astype(NPBF16)
    gam = np.ascontiguousarray(
        np.broadcast_to(np.asarray(ln_gamma, np.float32), (128, E)))
    bet = np.ascontiguousarray(
        np.broadcast_to(np.asarray(ln_beta, np.float32), (128, E)))

    nc = _get_nc()
    shared = {"dense_w": dense_w.astype(NPBF16), "b_col": b.reshape(E, 1),
              "b_row": b.reshape(1, E).astype(NPBF16), "w2": w2,
              "gamma_rep": gam, "beta_rep": bet}
    in_maps = [dict(shared, xT=np.ascontiguousarray(x[bi].T).astype(NPBF16))
               for bi in range(B)]
    res = run_bass_kernel_spmd(nc, in_maps, core_ids=list(range(B)))
    LAST_RESULT = res
    out = np.stack([res.results[bi]["out"] for bi in range(B)]).astype(np.float32)
    return out


if __name__ == "__main__":
    rng = np.random.default_rng(0)
    inputs = {
        "x": rng.standard_normal((B, S, E)).astype(np.float32),
        "dense_w": rng.standard_normal((E, E)).astype(np.float32) * 0.02,
        "dense_b": np.zeros(E, np.float32),
        "concept_map": rng.standard_normal((E, E, E)).astype(np.float32) * 0.02,
        "ln_gamma": np.ones(E, np.float32),
        "ln_beta": np.zeros(E, np.float32),
    }
    out = kernel(**inputs)
    print("out", out.shape, out.dtype, float(np.abs(out).max()))



# revision 2
# speedup vs baseline: 1.2702x; 1.2702x over previous
"""Trainium2 Bass kernel for nn_ConceptLayer (B=8, S=2048, E=128).

out[b,s,c] = LN( einsum('sa,sp,cap->sc', h[b], s_seq[b], W) + h[b] )
  h = x @ dense_w + dense_b
  s_seq = decayed prefix sum of h along s (s_i = (s_{i-1}+h_{i-1})/1.2)

Sharding: data-parallel over batch, one sample per NeuronCore (8 cores).

One-pass PE design (no big-Y intermediate, no per-a vector combine):
  outT[c, t] = sum_a W2[:, a-slice].T @ (sT * h[:,a])  accumulated in PSUM
             (+ Id.T @ hT for the residual)
Per 512-token group tg, per 16-wide a-batch ab:
  1. h broadcast: hT rows flattened to DRAM once, then one stride-0 DMA
     per 16 dst partitions replicates h[t,a] (fp16) to all 128 partitions
  2. DVE builds scaled operands: sc[p,(a,t)] = sT[p,t]*hB[p,(a,t)]
     (fp16 tensor_tensor, stride-0 a-dim on the sT operand, 2x mode)
  3. PE: 16 matmuls (lhsT=W2 a-slice, rhs=sc slice, N=512) accumulating
     into psumT[tg]; +1 identity matmul adds the residual h^T
  4. per 128-token block: PE transpose back to [t,c], LayerNorm
     (bn_stats/aggr on DVE, sqrt+apply on ACT, gamma/beta on GPSIMD)
"""

import dataclasses
import os
import sys

import numpy as np

for _p in ("/opt/trn_rl_repo",):
    if _p not in sys.path and os.path.isdir(_p):
        sys.path.insert(0, _p)

import concourse.bass as bass
import concourse.bacc as bacc
import concourse.tile as tile
from concourse import mybir
from concourse.bass_utils import run_bass_kernel_spmd

B, S, E = 8, 2048, 128
DECAY = 1.2
LN_EPS = 1e-3
NTG = 4          # token groups of 512
TGW = S // NTG   # 512
NAB = 8          # a-batches of 16 per token group
ABW = E // NAB   # 16
NBLK = S // 128  # 16 token blocks (for transpose+LN)
F32 = mybir.dt.float32
F16 = mybir.dt.float16
NPF16 = mybir.dt.np(F16)

_CACHE = {}
LAST_RESULT = None  # BassKernelResults of the most recent run (for test.py)


def _build_nc():
    nc = bacc.Bacc(None, target_bir_lowering=False)

    xT_d = nc.declare_dram_parameter("xT", [E, S], F16, isOutput=False)
    dw_d = nc.declare_dram_parameter("dense_w", [E, E], F16, isOutput=False)
    bc_d = nc.declare_dram_parameter("b_col", [E, 1], F32, isOutput=False)
    bcd_d = nc.declare_dram_parameter("b_col_d", [E, 1], F32, isOutput=False)
    w2_d = nc.declare_dram_parameter("w2", [E, E * E], F16, isOutput=False)
    id_d = nc.declare_dram_parameter("id128", [E, E], F16, isOutput=False)
    gam_d = nc.declare_dram_parameter("gamma_rep", [128, E], F32, isOutput=False)
    bet_d = nc.declare_dram_parameter("beta_rep", [128, E], F32, isOutput=False)
    out_d = nc.declare_dram_parameter("out", [S, E], F32, isOutput=True)

    hsc_d = nc.dram_tensor("hscratch", [NTG * NAB, ABW * TGW], F16, kind="Internal")

    with tile.TileContext(nc) as tc:
        with (
            tc.tile_pool(name="singles", bufs=1) as singles,
            tc.tile_pool(name="hb", bufs=2) as hb_pool,
            tc.tile_pool(name="sc", bufs=2) as sc_pool,
            tc.tile_pool(name="ln", bufs=2) as ln_pool,
            tc.tile_pool(name="small", bufs=4) as small,
            tc.tile_pool(name="h_ps", bufs=2, space="PSUM") as h_ps,
            tc.tile_pool(name="mm_ps", bufs=2, space="PSUM") as mm_ps,
            tc.tile_pool(name="tr_ps", bufs=2, space="PSUM") as tr_ps,
        ):
            # ---- resident tensors ----
            w2_sb = singles.tile([E, E * E], F16)
            nc.sync.dma_start(out=w2_sb[:], in_=w2_d[:])
            dw_sb = singles.tile([E, E], F16)
            nc.sync.dma_start(out=dw_sb[:], in_=dw_d[:])
            id_sb = singles.tile([E, E], F16)
            nc.sync.dma_start(out=id_sb[:], in_=id_d[:])
            bcol = singles.tile([E, 1], F32)
            nc.sync.dma_start(out=bcol[:], in_=bc_d[:])
            bcold = singles.tile([E, 1], F32)
            nc.sync.dma_start(out=bcold[:], in_=bcd_d[:])
            gam_sb = singles.tile([128, E], F32)
            nc.sync.dma_start(out=gam_sb[:], in_=gam_d[:])
            bet_sb = singles.tile([128, E], F32)
            nc.sync.dma_start(out=bet_sb[:], in_=bet_d[:])
            xT = singles.tile([E, S], F16)
            nc.sync.dma_start(out=xT[:], in_=xT_d[:])

            eps_t = singles.tile([128, 1], F32)
            nc.vector.memset(eps_t[:], LN_EPS)
            dinv = singles.tile([128, TGW], F16)
            nc.vector.memset(dinv[:], 1.0 / DECAY)

            hTc = singles.tile([E, S], F16)      # h^T (with bias), clean
            hTp = singles.tile([E, S + 1], F16)  # scan feed: col j+1 = (h_j+b)/d
            sTmm = singles.tile([E, S], F16)     # decayed prefix sums s^T
            nc.vector.memset(hTp[:, 0:1], 0.0)

            # ---- phase A: hT = dw^T @ xT (+bias); scan feed ----
            for q in range(NTG):
                lo, hi = q * TGW, (q + 1) * TGW
                hp = h_ps.tile([E, TGW], F32)
                nc.tensor.matmul(hp[:], dw_sb[:], xT[:, lo:hi],
                                 start=True, stop=True)
                nc.vector.tensor_scalar(
                    hTc[:, lo:hi], hp[:], bcol[:], None, mybir.AluOpType.add)
                nc.scalar.activation(
                    hTp[:, lo + 1:hi + 1], hp[:],
                    mybir.ActivationFunctionType.Identity,
                    bias=bcold[:], scale=1.0 / DECAY)

            # ---- phase B: decay prefix scan -> sT (fp16) ----
            for q in range(NTG):
                lo, hi = q * TGW, (q + 1) * TGW
                init = 0.0 if q == 0 else sTmm[:, lo - 1:lo]
                nc.vector.tensor_tensor_scan(
                    sTmm[:, lo:hi], dinv[:], hTp[:, lo:hi], init,
                    mybir.AluOpType.mult, mybir.AluOpType.add)

            # ---- phase A.5: flatten hT tiles to DRAM for broadcast ----
            for tg in range(NTG):
                for ab in range(NAB):
                    idx = tg * NAB + ab
                    nc.sync.dma_start(
                        out=hsc_d[idx:idx + 1, :],
                        in_=hTc[ab * ABW:(ab + 1) * ABW,
                                tg * TGW:(tg + 1) * TGW])

            # ---- main loop ----
            for tg in range(NTG):
                lo, hi = tg * TGW, (tg + 1) * TGW
                psumT = mm_ps.tile([E, TGW], F32)
                # residual: outT += Id^T @ hT  (starts the accumulation)
                nc.tensor.matmul(psumT[:], id_sb[:], hTc[:, lo:hi],
                                 start=True, stop=False, skip_group_check=True)
                for ab in range(NAB):
                    idx = tg * NAB + ab
                    # broadcast h[t,a] (a-major flat) to all 128 partitions
                    hB = hb_pool.tile([128, ABW, TGW], F16)
                    src = hsc_d[idx:idx + 1, :]
                    for j in range(8):
                        bsrc = dataclasses.replace(
                            src, ap=[[0, 16], [1, ABW * TGW]])
                        nc.sync.dma_start(out=hB[16 * j:16 * (j + 1), :, :],
                                          in_=bsrc)
                    # scaled operands: sc[p,(a,t)] = sT[p,t] * h[t,a]
                    sc = sc_pool.tile([128, ABW, TGW], F16)
                    in0 = sTmm[:, lo:hi]
                    in0 = dataclasses.replace(
                        in0, ap=[in0.ap[0], [0, ABW]] + in0.ap[1:])
                    nc.vector.tensor_tensor(out=sc[:, :, :], in0=in0,
                                            in1=hB[:, :, :],
                                            op=mybir.AluOpType.mult)
                    # PE: accumulate sum_a W2_a^T @ sc_a into psumT
                    for i in range(ABW):
                        a = ab * ABW + i
                        last = (ab == NAB - 1) and (i == ABW - 1)
                        nc.tensor.matmul(
                            psumT[:], w2_sb[:, a * E:(a + 1) * E], sc[:, i, :],
                            start=False, stop=last, skip_group_check=True)

                # drain psumT -> SBUF fp16 (ACT), transposed back per block + LN
                rT = ln_pool.tile([E, TGW], F16)
                nc.scalar.copy(out=rT[:], in_=psumT[:])
                for blk in range(TGW // 128):
                    g = tg * (TGW // 128) + blk
                    rtp = tr_ps.tile([128, 128], F16)
                    nc.tensor.transpose(
                        rtp[:], rT[:, blk * 128:(blk + 1) * 128], id_sb[:])
                    stats = small.tile([128, 6], F32)
                    nc.vector.bn_stats(out=stats[:], in_=rtp[:])
                    mv = small.tile([128, 2], F32)
                    nc.vector.bn_aggr(out=mv[:], in_=stats[:])
                    std = small.tile([128, 1], F32)
                    nc.scalar.activation(out=std[:], in_=mv[:, 1:2],
                                         func=mybir.ActivationFunctionType.Sqrt,
                                         bias=eps_t[:], scale=1.0)
                    rstd = small.tile([128, 1], F32)
                    nc.vector.reciprocal(out=rstd[:], in_=std[:])
                    nbias = small.tile([128, 1], F32)
                    nc.vector.tensor_scalar(
                        nbias[:], mv[:, 0:1], rstd[:], -1.0,
                        mybir.AluOpType.mult, mybir.AluOpType.mult)
                    nrm = ln_pool.tile([128, E], F32)
                    nc.scalar.activation(
                        out=nrm[:], in_=rtp[:],
                        func=mybir.ActivationFunctionType.Identity,
                        bias=nbias[:], scale=rstd[:])
                    nc.gpsimd.tensor_tensor(out=nrm[:], in0=nrm[:],
                                            in1=gam_sb[:],
                                            op=mybir.AluOpType.mult)
                    nc.gpsimd.tensor_tensor(out=nrm[:], in0=nrm[:],
                                            in1=bet_sb[:],
                                            op=mybir.AluOpType.add)
                    nc.sync.dma_start(out=out_d[g * 128:(g + 1) * 128, :],
                                      in_=nrm[:])

    nc.compile()
    return nc


def _get_nc():
    if "nc" not in _CACHE:
        _CACHE["nc"] = _build_nc()
    return _CACHE["nc"]


def kernel(x, dense_w, dense_b, concept_map, ln_gamma, ln_beta):
    global LAST_RESULT
    x = np.asarray(x, dtype=np.float32)
    dense_w = np.ascontiguousarray(np.asarray(dense_w, dtype=np.float32))
    b = np.asarray(dense_b, dtype=np.float32)
    w2 = np.ascontiguousarray(
        np.transpose(np.asarray(concept_map, dtype=np.float32), (2, 1, 0))
    ).reshape(E, E * E).astype(NPF16)
    gam = np.ascontiguousarray(
        np.broadcast_to(np.asarray(ln_gamma, np.float32), (128, E)))
    bet = np.ascontiguousarray(
        np.broadcast_to(np.asarray(ln_beta, np.float32), (128, E)))

    nc = _get_nc()
    shared = {
        "dense_w": dense_w.astype(NPF16),
        "b_col": b.reshape(E, 1),
        "b_col_d": (b / DECAY).reshape(E, 1),
        "w2": w2,
        "id128": np.eye(E, dtype=NPF16),
        "gamma_rep": gam,
        "beta_rep": bet,
    }
    in_maps = [dict(shared, xT=np.ascontiguousarray(x[bi].T).astype(NPF16))
               for bi in range(B)]
    res = run_bass_kernel_spmd(nc, in_maps, core_ids=list(range(B)))
    LAST_RESULT = res
    out = np.stack([res.results[bi]["out"] for bi in range(B)]).astype(np.float32)
    return out


if __name__ == "__main__":
    rng = np.random.default_rng(0)
    inputs = {
        "x": rng.standard_normal((B, S, E)).astype(np.float32),
        "dense_w": rng.standard_normal((E, E)).astype(np.float32) * 0.02,
        "dense_b": np.zeros(E, np.float32),
        "concept_map": rng.standard_normal((E, E, E)).astype(np.float32) * 0.02,
        "ln_gamma": np.ones(E, np.float32),
        "ln_beta": np.zeros(E, np.float32),
    }
    out = kernel(**inputs)
    print("out", out.shape, out.dtype, float(np.abs(out).max()))
